# revision 25
# baseline (speedup 1.0000x reference)
"""Trainium2 Bass kernel for nn_AdaptativeGCN (gnn_message_passing).

Computation (reference):
    sec   = relu(A @ (X Ws1) + X Ws2 + bs)                 [N, 32]
    S     = [sec | P]                                      [N, 96]
    msec  = A @ (S Wm2a) + S Wm2b + bm2                    [N, 7]
    M     = [X | P]                                        [N, 192]
    main  = A @ (M Wma) + M Wmb + bm                       [N, 7]
    out   = softmax(0.5*(v2*main + v1*msec), axis=-1)      [N, 7]

v4 design (trace-driven rework of v3):
  * Padded-k layout: every rank's 1250 rows are padded to 1280 = 10 clean
    128-row tiles (zero pad in A^T), so the global contraction is 80
    uniform k-tiles = 40 DoubleRow pairs with no last-tile special case.
  * The A shard is uploaded in EXACT SBUF layout [128, 80*1280] fp8, so
    the AT stream is a pure linear copy: 2-ktile groups alternating on
    both HWDGE queues, 128 descriptors x 2560 B each (v3 used 512x1250B
    rearrange descriptors and started ~10us late at ~270 GB/s).
  * Gc bounce / gather readback / output all use partition-major layouts
    (160-280B descriptors). v3 used natural [rows, 7] layouts = storms of
    7-byte descriptors that added ~15us (bounce sems crawled 78->86us).
  * A tiny warm-up AllGather fires at t~0 to absorb rank start skew and
    ncfw cold-start, so the real Gc AllGather hits a hot control plane.
  * Dense (128-contraction) clock-keeper matmuls hold the PE p-state
    during the collective window; v3's 32-row junk let HAM halve the
    clock, costing ~2.5us at the start of pass 2b.

Distribution: row-shard A over 8 cores (1250 rows each). Both passes
over A (pass 1: (A X)^T with X k-tiles stationary; pass 2b: (A Gc)^T
with gathered Gc k-tiles stationary) stream the SBUF-resident fp8 shard
through the PE in DoubleRow mode.
"""

import sys
import types

import numpy as np


def _install_ntff_hook():
    """run_bass_kernel_spmd(trace=True) under axon needs antenv.axon_hooks,
    which the agent image lacks; register the ctypes-based hook ourselves."""
    try:
        from antenv.axon_hooks import get_axon_ntff_profile_hook  # noqa: F401
        return
    except ImportError:
        pass
    try:
        from trn_agent_boot.trn_boot import _ntff_profile_via_ctypes
        hook = _ntff_profile_via_ctypes('/opt/axon/libaxon_pjrt.so')
    except Exception:
        hook = None
    mod = types.ModuleType('antenv.axon_hooks')
    mod.get_axon_ntff_profile_hook = lambda: hook
    mod.set_axon_ntff_profile_hook = lambda h: None
    sys.modules['antenv.axon_hooks'] = mod


_install_ntff_hook()
if '/opt/trn_rl_repo' not in sys.path:
    sys.path.insert(0, '/opt/trn_rl_repo')

import os  # noqa: E402

import ml_dtypes  # noqa: E402
import concourse.bacc as bacc  # noqa: E402
import concourse.mybir as mybir  # noqa: E402
from concourse import masks, tile  # noqa: E402
from concourse.bass_utils import run_bass_kernel_spmd  # noqa: E402

BF16 = ml_dtypes.bfloat16
FP8 = ml_dtypes.float8_e4m3
NCORES = 8
N = 10000
F_T, F_P = 128, 64
SEC, MC = 32, 7
RL = N // NCORES            # local rows per core = 1250
RLP = 1280                  # padded rows per rank = 10 tiles of 128
LT = RLP // 128             # local k-tiles per rank = 10
KT = NCORES * LT            # global padded k-tiles = 80
NPAIR = KT // 2             # DoubleRow pairs = 40
GCP = 16                    # Gc stationary pitch (DoubleRow: step%16==0)
CHUNKS = [(0, 512), (512, 512), (1024, RL - 1024)]    # free-dim chunks
RC = [(i * 128, min(128, RL - i * 128)) for i in range(LT)]
ATGRP = int(os.environ.get("ATGRP", "2"))   # k-tiles per AT dma_start
JUNK = int(os.environ.get("JUNK", "48"))    # dense clock-keepers in CC window
WARMAG = os.environ.get("WARMAG", "1") == "1"
NWARM = int(os.environ.get("NWARM", "4"))   # early PE warm-up matmuls

_compiled = None


def _build():
    f32 = mybir.dt.float32
    bf16 = mybir.dt.bfloat16
    fp8 = mybir.dt.float8e4

    nc = bacc.Bacc("TRN2", target_bir_lowering=False, debug=False,
                   num_devices=NCORES)

    # ---- per-core inputs (at/xn already in SBUF layout, see host prep) ----
    at_e = nc.dram_tensor("at", [128, KT * RLP], fp8,
                          kind="ExternalInput").ap()
    xn_e = nc.dram_tensor("xn", [128, KT * 128], fp8,
                          kind="ExternalInput").ap()
    xtl_e = nc.dram_tensor("xtl", [F_T, RL], bf16, kind="ExternalInput").ap()
    pt_e = nc.dram_tensor("pt", [F_P, RL], bf16, kind="ExternalInput").ap()
    ws1_e = nc.dram_tensor("ws1", [F_T, SEC], bf16, kind="ExternalInput").ap()
    ws2_e = nc.dram_tensor("ws2", [F_T, SEC], bf16, kind="ExternalInput").ap()
    bs_e = nc.dram_tensor("bs", [SEC, 1], f32, kind="ExternalInput").ap()
    wgsp_e = nc.dram_tensor("wgsp", [SEC + F_P, MC], bf16,
                            kind="ExternalInput").ap()
    was_e = nc.dram_tensor("was", [SEC, MC], bf16, kind="ExternalInput").ap()
    wxa_e = nc.dram_tensor("wxa", [F_T, MC], bf16, kind="ExternalInput").ap()
    wxb_e = nc.dram_tensor("wxb", [F_T, MC], bf16, kind="ExternalInput").ap()
    wap_e = nc.dram_tensor("wap", [F_P, MC], bf16, kind="ExternalInput").ap()
    blb_e = nc.dram_tensor("blb", [MC, 512], f32, kind="ExternalInput").ap()
    out_e = nc.dram_tensor("out", [128, LT * MC], f32,
                           kind="ExternalOutput").ap()

    with tile.TileContext(nc) as tc:
        with (
            tc.tile_pool(name="const", bufs=1) as cp,
            tc.tile_pool(name="big", bufs=1) as bigp,
            tc.tile_pool(name="work", bufs=1) as wp,
            tc.tile_pool(name="psum", bufs=1, space="PSUM") as pp,
            tc.tile_pool(name="dram", bufs=1, space="DRAM") as dp,
        ):
            # ---- warm-up AllGather: fires immediately (input has no
            # writer), absorbing rank start-skew + ncfw cold start during
            # the AT stream so the real Gc gather hits a hot path ----
            if WARMAG:
                wag_in = dp.tile([128, 4], fp8, name="wag_in")
                wag_out = dp.tile([NCORES * 128, 4], fp8,
                                  addr_space="Shared", name="wag_out")
                nc.gpsimd.collective_compute(
                    "AllGather", mybir.AluOpType.bypass,
                    ins=[wag_in[:].opt()], outs=[wag_out[:].opt()],
                    replica_groups=[list(range(NCORES))],
                )

            # ---- constants / persistent tiles (small DMAs on gpsimd,
            # keeping both HWDGE queues clear for the AT stream) ----
            ws1_s = cp.tile([F_T, SEC], bf16)
            ws2_s = cp.tile([F_T, SEC], bf16)
            bs_s = cp.tile([SEC, 1], f32)
            wgsp_s = cp.tile([SEC + F_P, MC], bf16)
            was_s = cp.tile([SEC, MC], bf16)
            wxa_s = cp.tile([F_T, MC], bf16)
            wxb_s = cp.tile([F_T, MC], bf16)
            wap_s = cp.tile([F_P, MC], bf16)
            blb_s = cp.tile([MC, 512], f32)
            eye_s = cp.tile([MC, MC], f32)
            xtl_s = cp.tile([F_T, RL], bf16)
            pt_s = cp.tile([F_P, RL], bf16)
            xn_s = bigp.tile([128, KT * 128], fp8, name="xnfull")
            # [sec | P]^T stacked: rows 0:32 = relu output, 32:96 = P^T
            spt_s = bigp.tile([SEC + F_P, RL], bf16, name="spt")
            # xn head rides the sync HWDGE queue so the tensor queue's
            # first matmuls (and whatever the compiler parks behind them)
            # unblock at ~1.5us instead of ~14us
            nc.sync.dma_start(xn_s[:, 0:16 * 128], xn_e[:, 0:16 * 128])
            nc.gpsimd.dma_start(xn_s[:, 16 * 128:], xn_e[:, 16 * 128:])
            for dst, src in [(xtl_s, xtl_e), (pt_s, pt_e),
                             (ws1_s, ws1_e), (ws2_s, ws2_e), (bs_s, bs_e),
                             (wgsp_s, wgsp_e), (was_s, was_e),
                             (wxa_s, wxa_e), (wxb_s, wxb_e), (wap_s, wap_e),
                             (blb_s, blb_e)]:
                nc.gpsimd.dma_start(dst[:], src[:])
            nc.gpsimd.dma_start(spt_s[SEC:SEC + F_P, :], pt_e[:])
            masks.make_identity(nc, eye_s[:])

            gcl = bigp.tile([128, LT * GCP], fp8, name="gcl")
            # pad rows of the last local tile must be ZERO fp8 (not stale
            # psum bits): they multiply at3's zero pads in pass 2b, and a
            # NaN there would poison the accumulator (NaN * 0 = NaN)
            nc.gpsimd.memset(gcl[:], 0.0)

            axs = bigp.tile([128, RL], bf16, name="axs")     # (A X)^T bf16
            mainx = bigp.tile([MC, RL], f32, name="mainx")
            combT = bigp.tile([MC, RL], f32, name="combT")
            gcv_s = bigp.tile([128, KT * GCP], fp8, name="gcv")
            at_s = bigp.tile([128, KT * RLP], fp8, name="atcache")
            at3 = at_s[:].rearrange("p (k i) -> p k i", i=RLP)
            xn3 = xn_s[:].rearrange("p (k f) -> p k f", f=128)
            gcl3 = gcl[:].rearrange("p (t c) -> p t c", c=GCP)
            gcv3 = gcv_s[:].rearrange("p (k c) -> p k c", c=GCP)

            # ---- early PE warm-up on xn (ramps the HAM clock before
            # pass 1; only needs the first xn chunk) ----
            pxw = pp.tile([128, 512], f32, tag="warm", bufs=1, name="pxw")
            for wi in range(NWARM):
                nc.tensor.matmul(pxw[:, :], xn_s[:, wi * 128:wi * 128 + 128],
                                 xn_s[:, 0:512], start=True, stop=True)

            # ---- AT stream: linear SBUF-layout copy, ATGRP k-tiles per
            # dma_start, alternating across BOTH HWDGE queues. The last
            # 4 k-tiles go as SINGLE-tile DMAs: their completion sems
            # fire sooner, tightening pass 1's receipt-paced tail. ----
            for gi, g0 in enumerate(range(0, KT - 4, ATGRP)):
                g1 = min(g0 + ATGRP, KT - 4)
                eng = nc.sync if gi % 2 == 0 else nc.scalar
                eng.dma_start(
                    at_s[:, g0 * RLP:g1 * RLP], at_e[:, g0 * RLP:g1 * RLP])
            for si, kt in enumerate(range(KT - 4, KT)):
                eng = nc.scalar if si % 2 == 0 else nc.sync
                eng.dma_start(at_s[:, kt * RLP:(kt + 1) * RLP],
                              at_e[:, kt * RLP:(kt + 1) * RLP])

            # ---- pass 1: (A_loc @ X)^T, fp8 DoubleRow, X k-tiles
            # stationary. Last 2 pairs chunk-major so chunk 0's epilogue
            # starts ~2 pairs earlier. ----
            axt = [pp.tile([128, 512], f32, tag="acc", bufs=3, name=f"ax{i}")
                   for i in range(3)]
            for j in range(NPAIR - 2):
                for ci, (off, w) in enumerate(CHUNKS):
                    nc.tensor.matmul(axt[ci][:, :w], xn3[:, 2 * j:2 * j + 2, :],
                                     at3[:, 2 * j:2 * j + 2, off:off + w],
                                     start=(j == 0), stop=False,
                                     perf_mode=mybir.MatmulPerfMode.DoubleRow)
            ps_s = [pp.tile([SEC, 512], f32, tag="acc", bufs=3,
                            name=f"ps{i}") for i in range(3)]
            pgc = pp.tile([128, LT * GCP], f32, tag="gc", bufs=1, name="pgc")
            # last 2 pairs chunk-major: chunk 0 stops ~2 pairs early and
            # its psum copy (DVE) overlaps chunks 1-2's remaining matmuls
            for ci, (off, w) in enumerate(CHUNKS):
                for j in (NPAIR - 2, NPAIR - 1):
                    nc.tensor.matmul(axt[ci][:, :w], xn3[:, 2 * j:2 * j + 2, :],
                                     at3[:, 2 * j:2 * j + 2, off:off + w],
                                     start=False, stop=(j == NPAIR - 1),
                                     perf_mode=mybir.MatmulPerfMode.DoubleRow)
                # psum -> SBUF bf16 (feeds ws1-proj and the mainx term)
                nc.vector.tensor_copy(axs[:, off:off + w], axt[ci][:, :w])
            # sec pre-act: Ws1^T (A X)^T + Ws2^T Xloc^T, then relu on ACT
            for ci, (off, w) in enumerate(CHUNKS):
                nc.tensor.matmul(ps_s[ci][:, :w], ws1_s[:], axs[:, off:off + w],
                                 start=True, stop=False)
                nc.tensor.matmul(ps_s[ci][:, :w], ws2_s[:],
                                 xtl_s[:, off:off + w],
                                 start=False, stop=True)
                nc.scalar.activation(spt_s[0:SEC, off:off + w],
                                     ps_s[ci][:, :w],
                                     mybir.ActivationFunctionType.Relu,
                                     bias=bs_s[:, :])
            # Gc projection per row-tile into one psum bank: direct
            # [96, cw]-stationary matmuls. (A transposed variant with
            # fewer LDWEIGHTS was tried and lost ~10us: its PE<->DVE
            # ping-pong let the scheduler interleave the window-filler
            # matmuls ahead of the bounce chain.)
            for ri, (o2, cw) in enumerate(RC):
                nc.tensor.matmul(pgc[:cw, ri * GCP:ri * GCP + MC],
                                 spt_s[:, o2:o2 + cw], wgsp_s[:],
                                 start=True, stop=True)
            # copy only VALID rows into gcl (pad rows stay memset-zero),
            # STAGED per chunk's row-tiles with the bounce shipped in 3
            # parts: the early parts' DMA completion receipts (~3us each)
            # overlap the later parts' transfers, pulling the collective
            # trigger ~2.5-3us earlier
            pgc3 = pgc[:].rearrange("p (t c) -> p t c", c=GCP)
            gc_bounce = dp.tile([128, LT * GCP], fp8, name="gc_bounce")
            gc_gather = dp.tile([NCORES * 128, LT * GCP], fp8,
                                addr_space="Shared", name="gc_gather")
            for t0, t1 in [(0, 4), (4, 8), (8, 10)]:
                if t1 == LT:
                    nc.vector.tensor_copy(gcl3[:, t0:LT - 1, 0:MC],
                                          pgc3[:, t0:LT - 1, 0:MC])
                    nc.vector.tensor_copy(gcl3[:RC[-1][1], LT - 1, 0:MC],
                                          pgc3[:RC[-1][1], LT - 1, 0:MC])
                else:
                    nc.vector.tensor_copy(gcl3[:, t0:t1, 0:MC],
                                          pgc3[:, t0:t1, 0:MC])
                nc.sync.dma_start(gc_bounce[:, t0 * GCP:t1 * GCP],
                                  gcl[:, t0 * GCP:t1 * GCP])
            nc.gpsimd.collective_compute(
                "AllGather", mybir.AluOpType.bypass,
                ins=[gc_bounce[:].opt()], outs=[gc_gather[:].opt()],
                replica_groups=[list(range(NCORES))],
            )
            # readback split across BOTH HWDGE queues (scalar is idle
            # here): sync fetches ranks 0-1 (pass 2b's first pairs start
            # early, doubling as p-state ramp), scalar the rest
            gav = gc_gather[:].rearrange("(r p) x -> p r x", p=128)
            gcv2 = gcv_s[:].rearrange("p (r x) -> p r x", x=LT * GCP)
            nc.sync.dma_start(gcv2[:, 0:2, :], gav[:, 0:2, :])
            nc.scalar.dma_start(gcv2[:, 2:8, :], gav[:, 2:8, :])

            # ---- work that fills the collective window ----
            # local additive terms open the psum_main accumulation chain.
            # Each chunk opens with a gcl-reading dummy (closed, garbage)
            # whose rows 0:7 the was-matmul then OVERWRITES (start=True):
            # this fences ps_m behind the Gc chain so it cannot be
            # scheduled ahead of the bounce. gcl cols 7:16 are memset-0,
            # so the dummy's surviving rows 7:16 are exact zeros.
            ps_m = [pp.tile([SEC, 512], f32, tag="acc", bufs=3, name=f"pm{i}")
                    for i in range(3)]
            for ci, (off, w) in enumerate(CHUNKS):
                nc.tensor.matmul(ps_m[ci][:GCP, :w], gcl3[:, 0:2, :],
                                 at3[:, 0:2, off:off + w],
                                 start=True, stop=True,
                                 perf_mode=mybir.MatmulPerfMode.DoubleRow)
                nc.tensor.matmul(ps_m[ci][:MC, :w], was_s[:],
                                 spt_s[0:SEC, off:off + w],
                                 start=True, stop=False)
                nc.tensor.matmul(ps_m[ci][:MC, :w], wap_s[:],
                                 pt_s[:, off:off + w], start=False, stop=False)
            # dense clock-keepers: full-activity matmuls so HAM holds the
            # PE p-state through the collective window (one psum tile,
            # WAR-serialized on purpose). The FIRST one reads gcl so the
            # whole chain is fenced behind the bounce source — the
            # scheduler cannot slot junk ahead of the Gc chain.
            pj = pp.tile([128, 512], f32, tag="warm", bufs=1, name="pj")
            nc.tensor.matmul(pj[:GCP, :], gcl3[:, 0:2, :],
                             at3[:, 0:2, 0:512], start=True, stop=True,
                             perf_mode=mybir.MatmulPerfMode.DoubleRow)
            for ji in range(JUNK - 1):
                jj = ji % (NPAIR - 1)
                nc.tensor.matmul(pj[:, :], xn3[:, 2 * jj:2 * jj + 2, :],
                                 at3[:, 2 * jj:2 * jj + 2, 0:512],
                                 start=True, stop=True,
                                 perf_mode=mybir.MatmulPerfMode.DoubleRow)
            # main-X local term AFTER the junk chain: same "warm" pool
            # slot, so px's first write WARs on pj — it cannot interleave
            # into the Gc/bounce chain and instead fills the window tail
            px = [pp.tile([MC, 512], f32, tag="warm", bufs=1, name=f"px{i}")
                  for i in range(3)]
            for ci, (off, w) in enumerate(CHUNKS):
                nc.tensor.matmul(px[ci][:, :w], wxa_s[:], axs[:, off:off + w],
                                 start=True, stop=False)
                nc.tensor.matmul(px[ci][:, :w], wxb_s[:], xtl_s[:, off:off + w],
                                 start=False, stop=True)
                nc.vector.tensor_add(mainx[:, off:off + w], px[ci][:, :w],
                                     blb_s[:, :w])

            # ---- pass 2b: += (A @ Gc)_loc^T, AT straight from SBUF.
            # Chunk-major: chunk ci's epilogue overlaps later chunks. ----
            ptl = pp.tile([128, LT * MC], f32, tag="ptile", bufs=1, name="ptl")
            ex = wp.tile([128, LT * MC], f32, name="ex")
            sm = wp.tile([128, LT], f32, name="sm")
            rcp = wp.tile([128, LT], f32, name="rcp")
            ot = wp.tile([128, LT * MC], f32, name="ot")
            ex3 = ex[:].rearrange("p (g c) -> p g c", c=MC)
            ot3 = ot[:].rearrange("p (g c) -> p g c", c=MC)
            GPC = [(0, 4), (4, 8), (8, 10)]   # row-tile groups per chunk
            for ci, (off, w) in enumerate(CHUNKS):
                for j in range(NPAIR):
                    nc.tensor.matmul(ps_m[ci][:GCP, :w],
                                     gcv3[:, 2 * j:2 * j + 2, :],
                                     at3[:, 2 * j:2 * j + 2, off:off + w],
                                     start=False, stop=(j == NPAIR - 1),
                                     perf_mode=mybir.MatmulPerfMode.DoubleRow)
                nc.vector.tensor_add(combT[:, off:off + w], ps_m[ci][:MC, :w],
                                     mainx[:, off:off + w])
                for ri, (o2, cw) in enumerate(RC):
                    if off <= o2 < off + w:
                        nc.tensor.transpose(ptl[:cw, ri * MC:(ri + 1) * MC],
                                            combT[:, o2:o2 + cw], eye_s[:])
                # per-chunk softmax + output: chunk ci's tail overlaps the
                # remaining chunks' matmuls; only chunk 2 (2 groups) is
                # serial at the very end
                g0, g1 = GPC[ci]
                nc.scalar.activation(ex[:, g0 * MC:g1 * MC],
                                     ptl[:, g0 * MC:g1 * MC],
                                     mybir.ActivationFunctionType.Exp)
                nc.vector.tensor_reduce(
                    sm[:, g0:g1], ex3[:, g0:g1, :],
                    axis=mybir.AxisListType.X, op=mybir.AluOpType.add)
                nc.vector.reciprocal(rcp[:, g0:g1], sm[:, g0:g1])
                nc.vector.tensor_mul(
                    ot3[:, g0:g1, :], ex3[:, g0:g1, :],
                    rcp[:, g0:g1].broadcast_to([128, g1 - g0, MC]))
                # partition-major output; host re-tiles to [1250, 7]
                nc.sync.dma_start(out_e[:, g0 * MC:g1 * MC],
                                  ot[:, g0 * MC:g1 * MC])

            # late tiny reader keeps the warm-up collective live
            if WARMAG:
                wscr = wp.tile([1, 4], fp8, name="wscr")
                nc.gpsimd.dma_start(wscr[:, :], wag_out[0:1, :])

    nc.compile()
    return nc


def _get_compiled():
    global _compiled
    if _compiled is None:
        _compiled = _build()
    return _compiled


def _pad_tiles(m, width):
    """[10000, width] -> [128, KT*width] in padded-k SBUF tile layout."""
    b = np.zeros((NCORES, RLP, width), m.dtype)
    b[:, :RL] = m.reshape(NCORES, RL, width)
    b = b.reshape(NCORES, LT, 128, width).transpose(2, 0, 1, 3)
    return np.ascontiguousarray(b.reshape(128, KT * width))


def kernel(temporal_features, A, path_features,
           Ws1, Ws2, bs, Wm2a, Wm2b, bm2, Wma, Wmb, bm, v1, v2,
           trace=False, tmpdir=None, trace_cores=None):
    nc = _get_compiled()

    X = np.asarray(temporal_features, np.float32)
    A = np.asarray(A, np.float32)
    P = np.asarray(path_features, np.float32)
    v1 = np.float32(v1)
    v2 = np.float32(v2)

    A8 = A.astype(FP8)
    xtf = np.ascontiguousarray(X.T)                        # [128, N] f32
    ptf = np.ascontiguousarray(P.T).astype(BF16)           # [64, N]
    xn = _pad_tiles(X, F_T).astype(FP8)                    # [128, KT*128]

    ws1 = np.asarray(Ws1, np.float32).astype(BF16)
    ws2 = np.asarray(Ws2, np.float32).astype(BF16)
    bs_in = np.asarray(bs, np.float32).reshape(SEC, 1)
    Wm2a = np.asarray(Wm2a, np.float32)
    Wm2b = np.asarray(Wm2b, np.float32)
    Wma = np.asarray(Wma, np.float32)
    Wmb = np.asarray(Wmb, np.float32)
    # pass-2 weights pre-scaled by 0.5*v (folds stack-mean + v-combine)
    wgsp = np.concatenate([
        0.5 * v1 * Wm2a[:SEC],
        0.5 * (v1 * Wm2a[SEC:] + v2 * Wma[F_T:]),
    ], axis=0).astype(BF16)
    was = (0.5 * v1 * Wm2b[:SEC]).astype(BF16)
    wxa = (0.5 * v2 * Wma[:F_T]).astype(BF16)
    wxb = (0.5 * v2 * Wmb[:F_T]).astype(BF16)
    wap = (0.5 * (v1 * Wm2b[SEC:] + v2 * Wmb[F_T:])).astype(BF16)
    bl = 0.5 * (v2 * np.asarray(bm, np.float32)
                + v1 * np.asarray(bm2, np.float32))
    blb = np.tile(bl.reshape(MC, 1), (1, 512)).astype(np.float32)

    in_maps = []
    for c in range(NCORES):
        r0, r1 = c * RL, (c + 1) * RL
        # at[p, kt*1280 + i] = A[r0+i, kpad(kt,p)], zero in the pad rows
        # and in the i-pitch pad (RL=1250 -> RLP=1280 per k-tile)
        t = _pad_tiles(np.ascontiguousarray(A8[r0:r1].T),
                       RL).reshape(128, KT, RL)
        at = np.zeros((128, KT, RLP), FP8)
        at[:, :, :RL] = t
        at = np.ascontiguousarray(at.reshape(128, KT * RLP))
        in_maps.append({
            "at": at,
            "xn": xn,
            "xtl": np.ascontiguousarray(xtf[:, r0:r1]).astype(BF16),
            "pt": np.ascontiguousarray(ptf[:, r0:r1]),
            "ws1": ws1, "ws2": ws2, "bs": bs_in,
            "wgsp": wgsp,
            "was": was, "wxa": wxa, "wxb": wxb, "wap": wap,
            "blb": blb,
        })

    kwargs = {}
    if trace_cores is not None:
        kwargs["trace_cores"] = trace_cores
    last_exc = None
    for attempt in range(3):
        try:
            res = run_bass_kernel_spmd(nc, in_maps, list(range(NCORES)),
                                       trace=trace, tmpdir=tmpdir, **kwargs)
            break
        except Exception as e:  # transient NRT device errors recover on retry
            last_exc = e
            import time as _time
            _time.sleep(3.0)
    else:
        raise last_exc
    outs = []
    for c in range(NCORES):
        o = res.results[c]["out"]                          # [128, LT*MC]
        outs.append(o.reshape(128, LT, MC).transpose(1, 0, 2)
                    .reshape(RLP, MC)[:RL])
    out = np.concatenate(outs, axis=0)
    kernel.last_result = res
    return out


# revision 26
# speedup vs baseline: 1.0645x; 1.0645x over previous
"""Trainium2 Bass kernel for nn_AdaptativeGCN (gnn_message_passing).

Computation (reference):
    sec   = relu(A @ (X Ws1) + X Ws2 + bs)                 [N, 32]
    S     = [sec | P]                                      [N, 96]
    msec  = A @ (S Wm2a) + S Wm2b + bm2                    [N, 7]
    M     = [X | P]                                        [N, 192]
    main  = A @ (M Wma) + M Wmb + bm                       [N, 7]
    out   = softmax(0.5*(v2*main + v1*msec), axis=-1)      [N, 7]

v4 design (trace-driven rework of v3):
  * Padded-k layout: every rank's 1250 rows are padded to 1280 = 10 clean
    128-row tiles (zero pad in A^T), so the global contraction is 80
    uniform k-tiles = 40 DoubleRow pairs with no last-tile special case.
  * The A shard is uploaded in EXACT SBUF layout [128, 80*1280] fp8, so
    the AT stream is a pure linear copy: 2-ktile groups alternating on
    both HWDGE queues, 128 descriptors x 2560 B each (v3 used 512x1250B
    rearrange descriptors and started ~10us late at ~270 GB/s).
  * Gc bounce / gather readback / output all use partition-major layouts
    (160-280B descriptors). v3 used natural [rows, 7] layouts = storms of
    7-byte descriptors that added ~15us (bounce sems crawled 78->86us).
  * A tiny warm-up AllGather fires at t~0 to absorb rank start skew and
    ncfw cold-start, so the real Gc AllGather hits a hot control plane.
  * Dense (128-contraction) clock-keeper matmuls hold the PE p-state
    during the collective window; v3's 32-row junk let HAM halve the
    clock, costing ~2.5us at the start of pass 2b.

Distribution: row-shard A over 8 cores (1250 rows each). Both passes
over A (pass 1: (A X)^T with X k-tiles stationary; pass 2b: (A Gc)^T
with gathered Gc k-tiles stationary) stream the SBUF-resident fp8 shard
through the PE in DoubleRow mode.
"""

import sys
import types

import numpy as np


def _install_ntff_hook():
    """run_bass_kernel_spmd(trace=True) under axon needs antenv.axon_hooks,
    which the agent image lacks; register the ctypes-based hook ourselves."""
    try:
        from antenv.axon_hooks import get_axon_ntff_profile_hook  # noqa: F401
        return
    except ImportError:
        pass
    try:
        from trn_agent_boot.trn_boot import _ntff_profile_via_ctypes
        hook = _ntff_profile_via_ctypes('/opt/axon/libaxon_pjrt.so')
    except Exception:
        hook = None
    mod = types.ModuleType('antenv.axon_hooks')
    mod.get_axon_ntff_profile_hook = lambda: hook
    mod.set_axon_ntff_profile_hook = lambda h: None
    sys.modules['antenv.axon_hooks'] = mod


_install_ntff_hook()
if '/opt/trn_rl_repo' not in sys.path:
    sys.path.insert(0, '/opt/trn_rl_repo')

import os  # noqa: E402

import ml_dtypes  # noqa: E402
import concourse.bacc as bacc  # noqa: E402
import concourse.mybir as mybir  # noqa: E402
from concourse import masks, tile  # noqa: E402
from concourse.bass_utils import run_bass_kernel_spmd  # noqa: E402

BF16 = ml_dtypes.bfloat16
FP8 = ml_dtypes.float8_e4m3
NCORES = 8
N = 10000
F_T, F_P = 128, 64
SEC, MC = 32, 7
RL = N // NCORES            # local rows per core = 1250
RLP = 1280                  # padded rows per rank = 10 tiles of 128
LT = RLP // 128             # local k-tiles per rank = 10
KT = NCORES * LT            # global padded k-tiles = 80
NPAIR = KT // 2             # DoubleRow pairs = 40
GCP = 16                    # Gc stationary pitch (DoubleRow: step%16==0)
CHUNKS = [(0, 512), (512, 512), (1024, RL - 1024)]    # free-dim chunks
RC = [(i * 128, min(128, RL - i * 128)) for i in range(LT)]
ATGRP = int(os.environ.get("ATGRP", "2"))   # k-tiles per AT dma_start
JUNK = int(os.environ.get("JUNK", "48"))    # dense clock-keepers in CC window
WARMAG = os.environ.get("WARMAG", "1") == "1"
NWARM = int(os.environ.get("NWARM", "4"))   # early PE warm-up matmuls

_compiled = None


def _build():
    f32 = mybir.dt.float32
    bf16 = mybir.dt.bfloat16
    fp8 = mybir.dt.float8e4

    nc = bacc.Bacc("TRN2", target_bir_lowering=False, debug=False,
                   num_devices=NCORES)

    # ---- per-core inputs (at/xn already in SBUF layout, see host prep) ----
    at_e = nc.dram_tensor("at", [128, KT * RLP], fp8,
                          kind="ExternalInput").ap()
    xn_e = nc.dram_tensor("xn", [128, KT * 128], fp8,
                          kind="ExternalInput").ap()
    xtl_e = nc.dram_tensor("xtl", [F_T, RL], bf16, kind="ExternalInput").ap()
    pt_e = nc.dram_tensor("pt", [F_P, RL], bf16, kind="ExternalInput").ap()
    ws1_e = nc.dram_tensor("ws1", [F_T, SEC], bf16, kind="ExternalInput").ap()
    ws2_e = nc.dram_tensor("ws2", [F_T, SEC], bf16, kind="ExternalInput").ap()
    bs_e = nc.dram_tensor("bs", [SEC, 1], f32, kind="ExternalInput").ap()
    wgsp_e = nc.dram_tensor("wgsp", [SEC + F_P, MC], bf16,
                            kind="ExternalInput").ap()
    was_e = nc.dram_tensor("was", [SEC, MC], bf16, kind="ExternalInput").ap()
    wxa_e = nc.dram_tensor("wxa", [F_T, MC], bf16, kind="ExternalInput").ap()
    wxb_e = nc.dram_tensor("wxb", [F_T, MC], bf16, kind="ExternalInput").ap()
    wap_e = nc.dram_tensor("wap", [F_P, MC], bf16, kind="ExternalInput").ap()
    blb_e = nc.dram_tensor("blb", [MC, 512], f32, kind="ExternalInput").ap()
    out_e = nc.dram_tensor("out", [128, LT * MC], f32,
                           kind="ExternalOutput").ap()

    with tile.TileContext(nc) as tc:
        with (
            tc.tile_pool(name="const", bufs=1) as cp,
            tc.tile_pool(name="big", bufs=1) as bigp,
            tc.tile_pool(name="work", bufs=1) as wp,
            tc.tile_pool(name="psum", bufs=1, space="PSUM") as pp,
            tc.tile_pool(name="dram", bufs=1, space="DRAM") as dp,
        ):
            # ---- warm-up AllGather: fires immediately (input has no
            # writer), absorbing rank start-skew + ncfw cold start during
            # the AT stream so the real Gc gather hits a hot path ----
            if WARMAG:
                wag_in = dp.tile([128, 4], fp8, name="wag_in")
                wag_out = dp.tile([NCORES * 128, 4], fp8,
                                  addr_space="Shared", name="wag_out")
                nc.gpsimd.collective_compute(
                    "AllGather", mybir.AluOpType.bypass,
                    ins=[wag_in[:].opt()], outs=[wag_out[:].opt()],
                    replica_groups=[list(range(NCORES))],
                )

            # ---- constants / persistent tiles (small DMAs on gpsimd,
            # keeping both HWDGE queues clear for the AT stream) ----
            ws1_s = cp.tile([F_T, SEC], bf16)
            ws2_s = cp.tile([F_T, SEC], bf16)
            bs_s = cp.tile([SEC, 1], f32)
            wgsp_s = cp.tile([SEC + F_P, MC], bf16)
            was_s = cp.tile([SEC, MC], bf16)
            wxa_s = cp.tile([F_T, MC], bf16)
            wxb_s = cp.tile([F_T, MC], bf16)
            wap_s = cp.tile([F_P, MC], bf16)
            blb_s = cp.tile([MC, 512], f32)
            eye_s = cp.tile([MC, MC], f32)
            xtl_s = cp.tile([F_T, RL], bf16)
            pt_s = cp.tile([F_P, RL], bf16)
            xn_s = bigp.tile([128, KT * 128], fp8, name="xnfull")
            # [sec | P]^T stacked: rows 0:32 = relu output, 32:96 = P^T
            spt_s = bigp.tile([SEC + F_P, RL], bf16, name="spt")
            # xn head rides the sync HWDGE queue so the tensor queue's
            # first matmuls (and whatever the compiler parks behind them)
            # unblock at ~1.5us instead of ~14us
            nc.sync.dma_start(xn_s[:, 0:16 * 128], xn_e[:, 0:16 * 128])
            nc.gpsimd.dma_start(xn_s[:, 16 * 128:], xn_e[:, 16 * 128:])
            for dst, src in [(xtl_s, xtl_e), (pt_s, pt_e),
                             (ws1_s, ws1_e), (ws2_s, ws2_e), (bs_s, bs_e),
                             (wgsp_s, wgsp_e), (was_s, was_e),
                             (wxa_s, wxa_e), (wxb_s, wxb_e), (wap_s, wap_e),
                             (blb_s, blb_e)]:
                nc.gpsimd.dma_start(dst[:], src[:])
            nc.gpsimd.dma_start(spt_s[SEC:SEC + F_P, :], pt_e[:])
            masks.make_identity(nc, eye_s[:])

            gcl = bigp.tile([128, LT * GCP], fp8, name="gcl")
            # pad rows of the last local tile must be ZERO fp8 (not stale
            # psum bits): they multiply at3's zero pads in pass 2b, and a
            # NaN there would poison the accumulator (NaN * 0 = NaN)
            nc.gpsimd.memset(gcl[:], 0.0)

            axs = bigp.tile([128, RL], bf16, name="axs")     # (A X)^T bf16
            mainx = bigp.tile([MC, RL], f32, name="mainx")
            combT = bigp.tile([MC, RL], f32, name="combT")
            gcv_s = bigp.tile([128, KT * GCP], fp8, name="gcv")
            at_s = bigp.tile([128, KT * RLP], fp8, name="atcache")
            at3 = at_s[:].rearrange("p (k i) -> p k i", i=RLP)
            xn3 = xn_s[:].rearrange("p (k f) -> p k f", f=128)
            gcl3 = gcl[:].rearrange("p (t c) -> p t c", c=GCP)
            gcv3 = gcv_s[:].rearrange("p (k c) -> p k c", c=GCP)

            # ---- early PE warm-up on xn (ramps the HAM clock before
            # pass 1; only needs the first xn chunk) ----
            pxw = pp.tile([128, 512], f32, tag="warm", bufs=1, name="pxw")
            for wi in range(NWARM):
                nc.tensor.matmul(pxw[:, :], xn_s[:, wi * 128:wi * 128 + 128],
                                 xn_s[:, 0:512], start=True, stop=True)

            # ---- AT stream: linear SBUF-layout copy, ATGRP k-tiles per
            # dma_start, alternating across BOTH HWDGE queues. The last
            # 4 k-tiles go as SINGLE-tile DMAs: their completion sems
            # fire sooner, tightening pass 1's receipt-paced tail. ----
            for gi, g0 in enumerate(range(0, KT - 4, ATGRP)):
                g1 = min(g0 + ATGRP, KT - 4)
                eng = nc.sync if gi % 2 == 0 else nc.scalar
                eng.dma_start(
                    at_s[:, g0 * RLP:g1 * RLP], at_e[:, g0 * RLP:g1 * RLP])
            for si, kt in enumerate(range(KT - 4, KT)):
                eng = nc.scalar if si % 2 == 0 else nc.sync
                eng.dma_start(at_s[:, kt * RLP:(kt + 1) * RLP],
                              at_e[:, kt * RLP:(kt + 1) * RLP])

            # ---- pass 1: (A_loc @ X)^T, fp8 DoubleRow, X k-tiles
            # stationary. Last 2 pairs chunk-major so chunk 0's epilogue
            # starts ~2 pairs earlier. ----
            axt = [pp.tile([128, 512], f32, tag="acc", bufs=3, name=f"ax{i}")
                   for i in range(3)]
            for j in range(NPAIR - 2):
                for ci, (off, w) in enumerate(CHUNKS):
                    nc.tensor.matmul(axt[ci][:, :w], xn3[:, 2 * j:2 * j + 2, :],
                                     at3[:, 2 * j:2 * j + 2, off:off + w],
                                     start=(j == 0), stop=False,
                                     perf_mode=mybir.MatmulPerfMode.DoubleRow)
            ps_s = [pp.tile([SEC, 512], f32, tag="acc", bufs=3,
                            name=f"ps{i}") for i in range(3)]
            pgc = pp.tile([128, LT * GCP], f32, tag="gc", bufs=1, name="pgc")
            # last 2 pairs chunk-major: chunk 0 stops ~2 pairs early and
            # its psum copy (DVE) overlaps chunks 1-2's remaining matmuls
            for ci, (off, w) in enumerate(CHUNKS):
                for j in (NPAIR - 2, NPAIR - 1):
                    nc.tensor.matmul(axt[ci][:, :w], xn3[:, 2 * j:2 * j + 2, :],
                                     at3[:, 2 * j:2 * j + 2, off:off + w],
                                     start=False, stop=(j == NPAIR - 1),
                                     perf_mode=mybir.MatmulPerfMode.DoubleRow)
                # psum -> SBUF bf16 (feeds ws1-proj and the mainx term)
                nc.vector.tensor_copy(axs[:, off:off + w], axt[ci][:, :w])
            # sec pre-act: Ws1^T (A X)^T + Ws2^T Xloc^T, then relu on ACT
            for ci, (off, w) in enumerate(CHUNKS):
                nc.tensor.matmul(ps_s[ci][:, :w], ws1_s[:], axs[:, off:off + w],
                                 start=True, stop=False)
                nc.tensor.matmul(ps_s[ci][:, :w], ws2_s[:],
                                 xtl_s[:, off:off + w],
                                 start=False, stop=True)
                nc.scalar.activation(spt_s[0:SEC, off:off + w],
                                     ps_s[ci][:, :w],
                                     mybir.ActivationFunctionType.Relu,
                                     bias=bs_s[:, :])
            # Gc projection per row-tile into one psum bank: direct
            # [96, cw]-stationary matmuls. (A transposed variant with
            # fewer LDWEIGHTS was tried and lost ~10us: its PE<->DVE
            # ping-pong let the scheduler interleave the window-filler
            # matmuls ahead of the bounce chain.)
            for ri, (o2, cw) in enumerate(RC):
                nc.tensor.matmul(pgc[:cw, ri * GCP:ri * GCP + MC],
                                 spt_s[:, o2:o2 + cw], wgsp_s[:],
                                 start=True, stop=True)
            # copy only VALID rows into gcl (pad rows stay memset-zero)
            pgc3 = pgc[:].rearrange("p (t c) -> p t c", c=GCP)
            nc.vector.tensor_copy(gcl3[:, 0:LT - 1, 0:MC],
                                  pgc3[:, 0:LT - 1, 0:MC])
            nc.vector.tensor_copy(gcl3[:RC[-1][1], LT - 1, 0:MC],
                                  pgc3[:RC[-1][1], LT - 1, 0:MC])

            # ---- bounce -> AllGather -> readback, all partition-major ----
            gc_bounce = dp.tile([128, LT * GCP], fp8, name="gc_bounce")
            gc_gather = dp.tile([NCORES * 128, LT * GCP], fp8,
                                addr_space="Shared", name="gc_gather")
            nc.sync.dma_start(gc_bounce[:, :], gcl[:, :])
            nc.gpsimd.collective_compute(
                "AllGather", mybir.AluOpType.bypass,
                ins=[gc_bounce[:].opt()], outs=[gc_gather[:].opt()],
                replica_groups=[list(range(NCORES))],
            )
            # readback split across BOTH HWDGE queues (scalar is idle
            # here): sync fetches ranks 0-1 (pass 2b's first pairs start
            # early, doubling as p-state ramp), scalar the rest
            gav = gc_gather[:].rearrange("(r p) x -> p r x", p=128)
            gcv2 = gcv_s[:].rearrange("p (r x) -> p r x", x=LT * GCP)
            nc.sync.dma_start(gcv2[:, 0:2, :], gav[:, 0:2, :])
            nc.scalar.dma_start(gcv2[:, 2:8, :], gav[:, 2:8, :])

            # ---- work that fills the collective window ----
            # local additive terms open the psum_main accumulation chain.
            # Each chunk opens with a gcl-reading dummy (closed, garbage)
            # whose rows 0:7 the was-matmul then OVERWRITES (start=True):
            # this fences ps_m behind the Gc chain so it cannot be
            # scheduled ahead of the bounce. gcl cols 7:16 are memset-0,
            # so the dummy's surviving rows 7:16 are exact zeros.
            ps_m = [pp.tile([SEC, 512], f32, tag="acc", bufs=3, name=f"pm{i}")
                    for i in range(3)]
            for ci, (off, w) in enumerate(CHUNKS):
                nc.tensor.matmul(ps_m[ci][:GCP, :w], gcl3[:, 0:2, :],
                                 at3[:, 0:2, off:off + w],
                                 start=True, stop=True,
                                 perf_mode=mybir.MatmulPerfMode.DoubleRow)
                nc.tensor.matmul(ps_m[ci][:MC, :w], was_s[:],
                                 spt_s[0:SEC, off:off + w],
                                 start=True, stop=False)
                nc.tensor.matmul(ps_m[ci][:MC, :w], wap_s[:],
                                 pt_s[:, off:off + w], start=False, stop=False)
            # dense clock-keepers: full-activity matmuls so HAM holds the
            # PE p-state through the collective window (one psum tile,
            # WAR-serialized on purpose). The FIRST one reads gcl so the
            # whole chain is fenced behind the bounce source — the
            # scheduler cannot slot junk ahead of the Gc chain.
            pj = pp.tile([128, 512], f32, tag="warm", bufs=1, name="pj")
            nc.tensor.matmul(pj[:GCP, :], gcl3[:, 0:2, :],
                             at3[:, 0:2, 0:512], start=True, stop=True,
                             perf_mode=mybir.MatmulPerfMode.DoubleRow)
            for ji in range(JUNK - 1):
                jj = ji % (NPAIR - 1)
                nc.tensor.matmul(pj[:, :], xn3[:, 2 * jj:2 * jj + 2, :],
                                 at3[:, 2 * jj:2 * jj + 2, 0:512],
                                 start=True, stop=True,
                                 perf_mode=mybir.MatmulPerfMode.DoubleRow)
            # main-X local term AFTER the junk chain: same "warm" pool
            # slot, so px's first write WARs on pj — it cannot interleave
            # into the Gc/bounce chain and instead fills the window tail
            px = [pp.tile([MC, 512], f32, tag="warm", bufs=1, name=f"px{i}")
                  for i in range(3)]
            for ci, (off, w) in enumerate(CHUNKS):
                nc.tensor.matmul(px[ci][:, :w], wxa_s[:], axs[:, off:off + w],
                                 start=True, stop=False)
                nc.tensor.matmul(px[ci][:, :w], wxb_s[:], xtl_s[:, off:off + w],
                                 start=False, stop=True)
                nc.vector.tensor_add(mainx[:, off:off + w], px[ci][:, :w],
                                     blb_s[:, :w])

            # ---- pass 2b: += (A @ Gc)_loc^T, AT straight from SBUF.
            # Chunk-major: chunk ci's epilogue overlaps later chunks. ----
            ptl = pp.tile([128, LT * MC], f32, tag="ptile", bufs=1, name="ptl")
            ex = wp.tile([128, LT * MC], f32, name="ex")
            sm = wp.tile([128, LT], f32, name="sm")
            rcp = wp.tile([128, LT], f32, name="rcp")
            ot = wp.tile([128, LT * MC], f32, name="ot")
            ex3 = ex[:].rearrange("p (g c) -> p g c", c=MC)
            ot3 = ot[:].rearrange("p (g c) -> p g c", c=MC)
            GPC = [(0, 4), (4, 8), (8, 10)]   # row-tile groups per chunk
            for ci, (off, w) in enumerate(CHUNKS):
                for j in range(NPAIR):
                    nc.tensor.matmul(ps_m[ci][:GCP, :w],
                                     gcv3[:, 2 * j:2 * j + 2, :],
                                     at3[:, 2 * j:2 * j + 2, off:off + w],
                                     start=False, stop=(j == NPAIR - 1),
                                     perf_mode=mybir.MatmulPerfMode.DoubleRow)
                nc.vector.tensor_add(combT[:, off:off + w], ps_m[ci][:MC, :w],
                                     mainx[:, off:off + w])
                for ri, (o2, cw) in enumerate(RC):
                    if off <= o2 < off + w:
                        nc.tensor.transpose(ptl[:cw, ri * MC:(ri + 1) * MC],
                                            combT[:, o2:o2 + cw], eye_s[:])
                # per-chunk softmax + output: chunk ci's tail overlaps the
                # remaining chunks' matmuls; only chunk 2 (2 groups) is
                # serial at the very end
                g0, g1 = GPC[ci]
                nc.scalar.activation(ex[:, g0 * MC:g1 * MC],
                                     ptl[:, g0 * MC:g1 * MC],
                                     mybir.ActivationFunctionType.Exp)
                nc.vector.tensor_reduce(
                    sm[:, g0:g1], ex3[:, g0:g1, :],
                    axis=mybir.AxisListType.X, op=mybir.AluOpType.add)
                nc.vector.reciprocal(rcp[:, g0:g1], sm[:, g0:g1])
                nc.vector.tensor_mul(
                    ot3[:, g0:g1, :], ex3[:, g0:g1, :],
                    rcp[:, g0:g1].broadcast_to([128, g1 - g0, MC]))
                # partition-major output; host re-tiles to [1250, 7]
                nc.sync.dma_start(out_e[:, g0 * MC:g1 * MC],
                                  ot[:, g0 * MC:g1 * MC])

            # late tiny reader keeps the warm-up collective live
            if WARMAG:
                wscr = wp.tile([1, 4], fp8, name="wscr")
                nc.gpsimd.dma_start(wscr[:, :], wag_out[0:1, :])

    nc.compile()
    return nc


def _get_compiled():
    global _compiled
    if _compiled is None:
        _compiled = _build()
    return _compiled


def _pad_tiles(m, width):
    """[10000, width] -> [128, KT*width] in padded-k SBUF tile layout."""
    b = np.zeros((NCORES, RLP, width), m.dtype)
    b[:, :RL] = m.reshape(NCORES, RL, width)
    b = b.reshape(NCORES, LT, 128, width).transpose(2, 0, 1, 3)
    return np.ascontiguousarray(b.reshape(128, KT * width))


def kernel(temporal_features, A, path_features,
           Ws1, Ws2, bs, Wm2a, Wm2b, bm2, Wma, Wmb, bm, v1, v2,
           trace=False, tmpdir=None, trace_cores=None):
    nc = _get_compiled()

    X = np.asarray(temporal_features, np.float32)
    A = np.asarray(A, np.float32)
    P = np.asarray(path_features, np.float32)
    v1 = np.float32(v1)
    v2 = np.float32(v2)

    A8 = A.astype(FP8)
    xtf = np.ascontiguousarray(X.T)                        # [128, N] f32
    ptf = np.ascontiguousarray(P.T).astype(BF16)           # [64, N]
    xn = _pad_tiles(X, F_T).astype(FP8)                    # [128, KT*128]

    ws1 = np.asarray(Ws1, np.float32).astype(BF16)
    ws2 = np.asarray(Ws2, np.float32).astype(BF16)
    bs_in = np.asarray(bs, np.float32).reshape(SEC, 1)
    Wm2a = np.asarray(Wm2a, np.float32)
    Wm2b = np.asarray(Wm2b, np.float32)
    Wma = np.asarray(Wma, np.float32)
    Wmb = np.asarray(Wmb, np.float32)
    # pass-2 weights pre-scaled by 0.5*v (folds stack-mean + v-combine)
    wgsp = np.concatenate([
        0.5 * v1 * Wm2a[:SEC],
        0.5 * (v1 * Wm2a[SEC:] + v2 * Wma[F_T:]),
    ], axis=0).astype(BF16)
    was = (0.5 * v1 * Wm2b[:SEC]).astype(BF16)
    wxa = (0.5 * v2 * Wma[:F_T]).astype(BF16)
    wxb = (0.5 * v2 * Wmb[:F_T]).astype(BF16)
    wap = (0.5 * (v1 * Wm2b[SEC:] + v2 * Wmb[F_T:])).astype(BF16)
    bl = 0.5 * (v2 * np.asarray(bm, np.float32)
                + v1 * np.asarray(bm2, np.float32))
    blb = np.tile(bl.reshape(MC, 1), (1, 512)).astype(np.float32)

    in_maps = []
    for c in range(NCORES):
        r0, r1 = c * RL, (c + 1) * RL
        # at[p, kt*1280 + i] = A[r0+i, kpad(kt,p)], zero in the pad rows
        # and in the i-pitch pad (RL=1250 -> RLP=1280 per k-tile)
        t = _pad_tiles(np.ascontiguousarray(A8[r0:r1].T),
                       RL).reshape(128, KT, RL)
        at = np.zeros((128, KT, RLP), FP8)
        at[:, :, :RL] = t
        at = np.ascontiguousarray(at.reshape(128, KT * RLP))
        in_maps.append({
            "at": at,
            "xn": xn,
            "xtl": np.ascontiguousarray(xtf[:, r0:r1]).astype(BF16),
            "pt": np.ascontiguousarray(ptf[:, r0:r1]),
            "ws1": ws1, "ws2": ws2, "bs": bs_in,
            "wgsp": wgsp,
            "was": was, "wxa": wxa, "wxb": wxb, "wap": wap,
            "blb": blb,
        })

    kwargs = {}
    if trace_cores is not None:
        kwargs["trace_cores"] = trace_cores
    last_exc = None
    for attempt in range(3):
        try:
            res = run_bass_kernel_spmd(nc, in_maps, list(range(NCORES)),
                                       trace=trace, tmpdir=tmpdir, **kwargs)
            break
        except Exception as e:  # transient NRT device errors recover on retry
            last_exc = e
            import time as _time
            _time.sleep(3.0)
    else:
        raise last_exc
    outs = []
    for c in range(NCORES):
        o = res.results[c]["out"]                          # [128, LT*MC]
        outs.append(o.reshape(128, LT, MC).transpose(1, 0, 2)
                    .reshape(RLP, MC)[:RL])
    out = np.concatenate(outs, axis=0)
    kernel.last_result = res
    return out
